# revision 1
# baseline (speedup 1.0000x reference)
"""Trainium2 Bass kernel for the batched differentiable-Markowitz layer (v3).

FISTA on 256 batch rows/core (2 partition tiles), N=256, T=N_STEPS rounds.
Both tiles run the SAME step each round; instructions are emitted in
op-kind phases (all DVE stt's, then both ACT relu's, then the shared
[128,2] scalar chain, then PE transposes, then copies, then next matmuls)
so the strict-FIFO engine queues never park a ready instruction behind a
waiting one.

Recurrence (w~ unnormalized, isv_t = 1/sum(w~_t), all per-batch scalars
live in [128,2] tiles shared by both tiles - one DVE op per round):

  H~_t   = w~_{t-1} @ A                       PE, PSUM (2 matmuls)
  un_t   = (c_t isv_{t-2}) H~_{t-1} + lr p    DVE stt, scalar slot = s0 AP
  v_t    = ((1+c_t) isv_{t-1}) H~_t - un_t    DVE stt, scalar slot = s1 AP
  w~_t   = relu(v_t + thneg), S_t = accum     ACT (bias+accum fused)
  isv_t  = 1/max(S_t, floor)                  DVE [128,2]
  thneg -= (S_t - 1)/cnt                      lagged Newton, off-chain
  wta_t  = transpose(w~_t) -> PSUM -> SBUF    PE + ACT/DVE copies

Multiplicative normalization rides in the s0/s1 scalar APs (and makes the
lagged theta Newton stable - sum(w)=1 holds exactly).  lr*p enters as the
un-op's tensor operand; step 1 collapses to v_1 = H~_1 - lr*p since c_1=0.
t-schedule momentum capped at BETA_CAP.  lr from a 3-iter 128-column block
power iteration (bf16, max Rayleigh over columns, 1.10 safety).  bf16
matmuls for the first N_BF steps, float32r after.  Validated against the
step-exact numpy sim in sim2.py (T=12, k0=2: rel 2.5e-3 vs 2e-2 gate).

Sharding: data-parallel over batch, 256 rows/core, Sigma replicated.
"""

import math
from contextlib import ExitStack

import numpy as np

import concourse.bass as bass  # noqa: F401
import concourse.tile as tile
from concourse import bacc, mybir
from concourse.bass_utils import run_bass_kernel_spmd

F32 = mybir.dt.float32
F32R = mybir.dt.float32r
BF16 = mybir.dt.bfloat16
OP = mybir.AluOpType
RELU = mybir.ActivationFunctionType.Relu
COPY = mybir.ActivationFunctionType.Copy

N = 256
B_CORE = 256
N_CORES = 8
NB = B_CORE // 128
NK = N // 128

N_STEPS = 11
N_BF = 8
BETA_CAP = 0.55
K0_NEWTON = 3
POW_ITERS = 2
L_SAFETY = 1.15
CNT_EVERY = 4
S_FLOOR = 0.05


def _momentum_coeffs(n, cap):
    t = np.float32(1.0)
    cs = []
    for _ in range(n + 3):
        t_next = np.float32(0.5 * (1.0 + math.sqrt(1.0 + 4.0 * float(t) ** 2)))
        cs.append(min(float((t - np.float32(1.0)) / t_next), cap))
        t = t_next
    return cs


def _make_identity(nc, ap, base=0):
    nc.gpsimd.memset(ap, 0.0)
    nc.gpsimd.affine_select(
        out=ap, in_=ap, compare_op=OP.not_equal, fill=1.0, base=base,
        pattern=[[-1, ap.shape[1]]], channel_multiplier=1)


def markowitz_tile_kernel(tc, out_w, in_p, in_sig, *,
                          n_steps=N_STEPS, n_bf=N_BF, beta_cap=BETA_CAP,
                          k0=K0_NEWTON, pow_iters=POW_ITERS, safety=L_SAFETY,
                          cnt_every=CNT_EVERY):
    nc = tc.nc
    ctx = ExitStack()
    cs = _momentum_coeffs(n_steps, beta_cap)

    def phase_dt(t):
        return BF16 if t < n_bf else F32R

    const = ctx.enter_context(tc.tile_pool(name="const", bufs=1))
    vpool = ctx.enter_context(tc.tile_pool(name="v", bufs=4))
    wpool = ctx.enter_context(tc.tile_pool(name="w", bufs=5))
    rpool = ctx.enter_context(tc.tile_pool(name="r", bufs=5))
    wtpool = ctx.enter_context(tc.tile_pool(name="wt", bufs=6))
    xtpool = ctx.enter_context(tc.tile_pool(name="xt", bufs=4))
    ps_h = ctx.enter_context(tc.tile_pool(name="psh", bufs=2, space="PSUM"))
    ps_t = ctx.enter_context(tc.tile_pool(name="pst", bufs=2, space="PSUM"))
    ps_m = ctx.enter_context(tc.tile_pool(name="psm", bufs=2, space="PSUM"))

    with ctx:
        # ---- persistent state ----
        S = [const.tile([128, N], F32, name=f"S{k}") for k in range(NK)]
        P = [const.tile([128, N], F32, name=f"P{b}") for b in range(NB)]
        A = [const.tile([128, N], F32R, name=f"A{k}") for k in range(NK)]
        A_b = [const.tile([128, N], BF16, name=f"Ab{k}") for k in range(NK)]
        IA = [const.tile([128, N], F32, name=f"IA{k}") for k in range(NK)]
        ID_f = const.tile([128, 128], F32, name="IDf")
        ID = const.tile([128, 128], F32R, name="ID")
        ID_b = const.tile([128, 128], BF16, name="IDb")
        w0f = const.tile([128, N], F32, name="w0f")
        ONES = const.tile([128, 1], F32, name="ONES")
        # shared per-batch scalars: column b <-> tile b
        th2 = const.tile([128, 2], F32, name="th2")
        sv2 = const.tile([128, 2], F32, name="sv2")
        svm2 = const.tile([128, 2], F32, name="svm2")
        isv2 = const.tile([128, 2], F32, name="isv2")
        s1p2 = const.tile([128, 2], F32, name="s1p2")
        s0q2 = [const.tile([128, 2], F32, name=f"s0q2{j}") for j in range(2)]
        cv2 = const.tile([128, 2], F32, name="cv2")
        cc2 = const.tile([128, 2], F32, name="cc2")
        ic2 = const.tile([128, 2], F32, name="ic2")
        dl2 = const.tile([128, 2], F32, name="dl2")
        lr_vec = const.tile([128, 1], F32, name="lrv")
        nlr_vec = const.tile([128, 1], F32, name="nlrv")
        ray = const.tile([1, 128], F32, name="ray")
        ray_i = const.tile([1, 128], F32, name="rayi")
        lmax = const.tile([1, 1], F32, name="lmax")
        onesrow = const.tile([1, 128], F32, name="onesrow")
        ONES_b = const.tile([128, 1], BF16, name="ONESb")
        qs = const.tile([1, N], F32, name="qs")
        hrow = const.tile([1, N], F32, name="hrow")
        nlrN = const.tile([1, 1], F32, name="nlrN")
        nls = const.tile([1, 1], F32, name="nls")
        nlr_s = const.tile([1, 1], F32, name="nlrs")

        def thc(b):
            return th2[:, b:b + 1]

        def svc(b):
            return sv2[:, b:b + 1]

        # ---- load inputs ----
        for k in range(NK):
            nc.sync.dma_start(S[k][:], in_sig[128 * k:128 * (k + 1), :])
        for b in range(NB):
            nc.sync.dma_start(P[b][:], in_p[128 * b:128 * (b + 1), :])

        # ---- constants (no input deps) ----
        _make_identity(nc, ID_f[:])
        nc.vector.tensor_copy(ID[:], ID_f[:])
        nc.vector.tensor_copy(ID_b[:], ID_f[:])
        for k in range(NK):
            _make_identity(nc, IA[k][:], base=128 * k)
        nc.gpsimd.memset(ONES[:], 1.0)
        nc.gpsimd.memset(onesrow[:], 1.0)
        nc.vector.tensor_copy(ONES_b[:], ONES[:])
        nc.vector.memset(ic2[:], 1.0 / N)
        nc.vector.memset(s1p2[:], 1.0)

        # ---- power iteration (bf16, 128-col block, max Rayleigh) ----
        S_b = [const.tile([128, N], BF16, name=f"Sb{k}") for k in range(NK)]
        for k in range(NK):
            nc.vector.tensor_copy(S_b[k][:], S[k][:])
        qps = ps_m.tile([1, N], F32, tag="pps", name="qps")
        for k in range(NK):
            nc.tensor.matmul(qps[:], ONES_b[:], S_b[k][:],
                             start=(k == 0), stop=(k == NK - 1))
        nc.vector.tensor_copy(qs[:], qps[:])
        xc = [S_b[k][:, 0:128] for k in range(NK)]
        xp = None
        for it in range(pow_iters):
            xn = []
            for j in range(NK):
                px = ps_m.tile([128, 128], F32, tag="pps", name="pps")
                for k in range(NK):
                    nc.tensor.matmul(px[:], S_b[k][:, 128 * j:128 * (j + 1)],
                                     xc[k],
                                     start=(k == 0), stop=(k == NK - 1))
                xs = xtpool.tile([128, 128], BF16, tag="xs", name="xs")
                nc.vector.tensor_copy(xs[:], px[:])
                xn.append(xs)
            xp, xc = xc, [t[:] for t in xn]
        pnum = ps_m.tile([1, 128], F32, tag="pps", name="pps")
        pden = ps_m.tile([1, 128], F32, tag="pps", name="pps")
        for k in range(NK):
            prod_n = xtpool.tile([128, 128], F32, tag="prodn", name="prodn")
            prod_d = xtpool.tile([128, 128], F32, tag="prodd", name="prodd")
            nc.vector.tensor_tensor(prod_n[:], xc[k], xc[k], OP.mult)
            nc.vector.tensor_tensor(prod_d[:], xp[k], xc[k], OP.mult)
            nc.tensor.matmul(pnum[:], ONES[:], prod_n[:],
                             start=(k == 0), stop=(k == NK - 1))
            nc.tensor.matmul(pden[:], ONES[:], prod_d[:],
                             start=(k == 0), stop=(k == NK - 1))
        nc.vector.reciprocal(ray_i[:], pden[:])
        nc.vector.tensor_tensor(ray[:], pnum[:], ray_i[:], OP.mult)
        nc.vector.tensor_reduce(lmax[:], ray[:], axis=mybir.AxisListType.X,
                                op=OP.max)
        # nlr = -1/(safety*lmax); lr = -nlr
        nc.vector.tensor_scalar(nls[:], lmax[:], float(-safety), None, OP.mult)
        nc.vector.reciprocal(nlr_s[:], nls[:])
        nc.gpsimd.partition_broadcast(nlr_vec[:], nlr_s[:])
        nc.vector.tensor_scalar(lr_vec[:], nlr_vec[:], -1.0, None, OP.mult)
        nc.vector.tensor_scalar(nlrN[:], nlr_s[:], 1.0 / N, None, OP.mult)
        # H~_1 row: (1/N)(1 - lr*q)  (w_0 uniform makes H~_1 rank-1)
        nc.vector.tensor_scalar(hrow[:], qs[:], nlrN[:, 0:1], 1.0 / N,
                                OP.mult, OP.add)

        # ---- A = I - lr*Sigma;  P <- lr*p ----
        for k in range(NK):
            nc.vector.scalar_tensor_tensor(A[k][:], S[k][:], nlr_vec[:, 0:1],
                                           IA[k][:], op0=OP.mult, op1=OP.add)
            if n_bf > 0:
                nc.vector.tensor_copy(A_b[k][:], A[k][:])
        for b in range(NB):
            nc.vector.tensor_scalar(P[b][:], P[b][:], lr_vec[:, 0:1], None,
                                    OP.mult)

        # ---- iterate state ----
        wta = [None] * NB
        H_cur = [None] * NB
        H_prev = [None] * NB

        def mm_H(b, t):
            Amm = A_b if phase_dt(t - 1) == BF16 else A
            pw = ps_h.tile([128, N], F32, tag=f"psH{b}", name=f"psH{b}")
            for k in range(NK):
                nc.tensor.matmul(pw[:], wta[b][:, 128 * k:128 * (k + 1)],
                                 Amm[k][:],
                                 start=(k == 0), stop=(k == NK - 1))
            H_prev[b], H_cur[b] = H_cur[b], pw

        def round_step(t):
            """One FISTA step for BOTH tiles, phase-grouped emission."""
            dt_n = phase_dt(t)
            # A: momentum combine (DVE)
            vs = []
            for b in range(NB):
                v = vpool.tile([128, N], F32, tag="v", name="v")
                if t == 1:
                    # c_1 = 0: v_1 = H~_1 - lr p
                    nc.vector.scalar_tensor_tensor(
                        v[:], H_cur[b][:], 1.0, P[b][:],
                        op0=OP.mult, op1=OP.subtract)
                else:
                    un = vpool.tile([128, N], F32, tag="un", name="un")
                    nc.vector.scalar_tensor_tensor(
                        un[:], H_prev[b][:], s0q2[t % 2][:, b:b + 1], P[b][:],
                        op0=OP.mult, op1=OP.add)
                    nc.vector.scalar_tensor_tensor(
                        v[:], H_cur[b][:], s1p2[:, b:b + 1], un[:],
                        op0=OP.mult, op1=OP.subtract)
                vs.append(v)
            # B: relu + sum (ACT)
            wts = []
            for b in range(NB):
                wt = wpool.tile([128, N], dt_n if t < n_steps else F32,
                                tag="w", name="w")
                nc.scalar.activation(wt[:], vs[b][:], RELU, bias=thc(b),
                                     accum_out=svc(b))
                wts.append(wt)
            if t == n_steps:
                # final: normalize in row layout and DMA out
                nc.vector.reciprocal(isv2[:], sv2[:])
                for b in range(NB):
                    wf = rpool.tile([128, N], F32, tag="wf", name="wf")
                    nc.vector.tensor_scalar(wf[:], wts[b][:],
                                            isv2[:, b:b + 1], None, OP.mult)
                    nc.sync.dma_start(out_w[128 * b:128 * (b + 1), :], wf[:])
                return
            # D: transposes (PE) on unnormalized w~
            IDmm = ID_b if dt_n == BF16 else ID
            pts = []
            for b in range(NB):
                pt = ps_t.tile([128, N], dt_n, tag="psT", name="psT")
                for k in range(NK):
                    sl = slice(128 * k, 128 * (k + 1))
                    nc.tensor.transpose(pt[:, sl], wts[b][:, sl], IDmm[:])
                pts.append(pt)
            # E: copies PSUM->SBUF, emitted BEFORE the scalar chain so the
            # next matmuls are not parked behind it (tile0 DVE, tile1 ACT:
            # ACT is busy with wt1 exactly when copy0 becomes ready)
            for b in range(NB):
                nwa = wtpool.tile([128, N], dt_n, tag=f"wta{b}",
                                  name=f"wta{b}")
                nc.scalar.copy(nwa[:], pts[b][:])
                wta[b] = nwa
            # F: next-step matmuls (PE)
            for b in range(NB):
                mm_H(b, t + 1)
            # C: shared scalar chain (DVE, [128,2], a full round of slack)
            if t % cnt_every == 0 and t < n_steps:
                for b in range(NB):
                    m = rpool.tile([128, N], F32, tag="m", name="m")
                    nc.vector.tensor_scalar(m[:], wts[b][:], 0.0, None,
                                            OP.is_gt, OP.add,
                                            accum_out=cv2[:, b:b + 1])
            nc.vector.reciprocal(isv2[:], sv2[:])
            nc.vector.tensor_scalar(s1p2[:], isv2[:],
                                    float(1.0 + cs[t + 1]), None, OP.mult)
            if t + 2 <= n_steps:
                nc.vector.tensor_scalar(s0q2[t % 2][:], isv2[:],
                                        float(cs[t + 2]), None, OP.mult)
            nc.vector.scalar_tensor_tensor(dl2[:], sv2[:], 1.0, ic2[:],
                                           op0=OP.subtract, op1=OP.mult)
            nc.vector.tensor_tensor(th2[:], th2[:], dl2[:], OP.subtract)
            if t % cnt_every == 0 and t < n_steps:
                nc.vector.tensor_scalar(cc2[:], cv2[:], 1.0, None, OP.max)
                nc.vector.reciprocal(ic2[:], cc2[:])

        def cold_start():
            for b in range(NB):
                pw = ps_h.tile([128, N], F32, tag=f"psH{b}", name=f"psH{b}")
                nc.tensor.matmul(pw[:], onesrow[:], hrow[:],
                                 start=True, stop=True)
                H_cur[b] = pw
                H_prev[b] = pw
            nc.vector.memset(s0q2[0][:], float(cs[2]))   # step 2: isv_0 = 1
            # cold theta: all-active Newton on v_1 = H~_1 - lr p read via PSUM
            # (P part folded by biasless copy + explicit stt would cost ops;
            #  instead approximate with H~_1 alone is WRONG - do it right:)
            v1s = []
            for b in range(NB):
                v1 = vpool.tile([128, N], F32, tag="v", name="v")
                nc.vector.scalar_tensor_tensor(v1[:], H_cur[b][:], 1.0,
                                               P[b][:],
                                               op0=OP.mult, op1=OP.subtract)
                v1s.append(v1)
                scr = rpool.tile([128, N], F32, tag="r", name="r")
                nc.scalar.activation(scr[:], v1[:], COPY, accum_out=svc(b))
            nc.vector.tensor_scalar(th2[:], sv2[:], 1.0, -1.0 / N,
                                    OP.subtract, OP.mult)
            for it in range(k0):
                for b in range(NB):
                    r = rpool.tile([128, N], F32, tag="r", name="r")
                    nc.scalar.activation(r[:], v1s[b][:], RELU, bias=thc(b),
                                         accum_out=svc(b))
                    m = rpool.tile([128, N], F32, tag="m", name="m")
                    nc.vector.tensor_scalar(m[:], r[:], 0.0, None,
                                            OP.is_gt, OP.add,
                                            accum_out=cv2[:, b:b + 1])
                nc.vector.tensor_scalar(cc2[:], cv2[:], 1.0, None, OP.max)
                nc.vector.reciprocal(ic2[:], cc2[:])
                nc.vector.scalar_tensor_tensor(dl2[:], sv2[:], 1.0, ic2[:],
                                               op0=OP.subtract, op1=OP.mult)
                nc.vector.tensor_tensor(th2[:], th2[:], dl2[:], OP.subtract)

        cold_start()
        for t in range(1, n_steps + 1):
            round_step(t)


def build_nc(**kw):
    nc = bacc.Bacc("TRN2", target_bir_lowering=False, debug=False,
                   enable_asserts=False)
    p_in = nc.dram_tensor("p", [B_CORE, N], F32, kind="ExternalInput")
    s_in = nc.dram_tensor("sigma", [N, N], F32, kind="ExternalInput")
    w_out = nc.dram_tensor("w", [B_CORE, N], F32, kind="ExternalOutput")
    with tile.TileContext(nc) as tc:
        markowitz_tile_kernel(tc, w_out.ap(), p_in.ap(), s_in.ap(), **kw)
    nc.compile()
    return nc


_NC_CACHE = {}


def kernel(p_batch: np.ndarray, Sigma: np.ndarray, **kw) -> np.ndarray:
    B = p_batch.shape[0]
    rows = B // N_CORES
    assert rows == B_CORE and Sigma.shape == (N, N)
    key = tuple(sorted(kw.items()))
    if key not in _NC_CACHE:
        _NC_CACHE[key] = build_nc(**kw)
    nc = _NC_CACHE[key]
    p32 = np.ascontiguousarray(p_batch, dtype=np.float32)
    s32 = np.ascontiguousarray(Sigma, dtype=np.float32)
    in_maps = [{"p": p32[i * rows:(i + 1) * rows], "sigma": s32}
               for i in range(N_CORES)]
    res = run_bass_kernel_spmd(nc, in_maps, core_ids=list(range(N_CORES)))
    out = np.concatenate([r["w"] for r in res.results], axis=0)
    return out.astype(p_batch.dtype, copy=False)

